# revision 5
# baseline (speedup 1.0000x reference)
"""Trainium2 Bass kernel for nn_AnomalyDetector (multi-modal encoder + 2-layer
LSTM + normalizing flows + decoders + anomaly scores).

Data-parallel over 8 NeuronCores: batch 16384 -> 2048 per core. All on-chip
activations use a transposed layout [feature_on_partition, sample_on_free] so
the per-step LSTM matmuls need no transposes. Compute in bf16 with f32 PSUM
accumulation; outputs staged to one [830, 2048] f32 DRAM tensor per core.
"""

import numpy as np
import ml_dtypes

import concourse.bass as bass
import concourse.mybir as mybir
from concourse import bacc
from concourse.tile import TileContext
from concourse.bass import ts
from concourse.bass_utils import run_bass_kernel_spmd

BF16 = mybir.dt.bfloat16
F32 = mybir.dt.float32
AF = mybir.ActivationFunctionType

B = 16384
NCORES = 8
S = B // NCORES          # samples per core (2048)
T = 64
HID = 128
LAT = 64
DIMS = {"physical": 32, "orbital": 6, "signature": 256, "temporal": 16}
MODS = ["physical", "orbital", "signature", "temporal"]
NCH = 4                  # moving-sample chunks of 512
CH = S // NCH            # 512

# ------------------------------------------------------------------
# Output row map for the per-core OUT tensor [830, S] (feature, sample)
# ------------------------------------------------------------------
ROW_ENC = 0        # 4 x 64  (phys, orb, sig, temp)
ROW_TRF = 256      # 4 x 64
ROW_REC = 512      # 32, 6, 256, 16
REC_OFF = {"physical": 512, "orbital": 544, "signature": 550, "temporal": 806}
ROW_LD = 822       # 4 rows (phys, orb, sig, temp)
ROW_SC = 826       # 4 rows
NROWS = 830

# ------------------------------------------------------------------
# Weight blob (bf16) / param blob (f32) layouts: name -> (r0, nr, c0, nc)
# ------------------------------------------------------------------
def _build_wspec():
    spec = {}
    col = 0

    def add(name, r0, nr, nc):
        nonlocal col
        spec[name] = (r0, nr, col, nc)
        col += nc

    add("wih1", 0, 128, 512)      # 4x replicated along 32-row groups
    add("whh1", 0, 128, 512)
    add("wih2", 0, 128, 512)
    add("whh2", 0, 128, 512)
    add("wtp", 0, 128, 64)
    add("e1p", 0, 32, 128)
    add("e2p", 0, 128, 64)
    add("e1o", 0, 6, 128)
    add("e2o", 0, 128, 64)
    add("e1s_a", 0, 128, 128)
    add("e1s_b", 0, 128, 128)
    add("e2s", 0, 128, 64)
    add("fw0", 0, 128, 128)
    add("fw1", 0, 128, 128)
    add("fw2", 0, 128, 128)
    add("ones2", 0, 128, 2)
    add("d1p", 0, 64, 128)
    add("d1o", 64, 64, 128)
    add("d1s", 0, 64, 128)
    add("d1t", 64, 64, 128)
    add("d2p", 0, 128, 32)
    add("d2o", 0, 128, 6)
    add("d2s", 0, 128, 256)
    add("d2t", 0, 128, 16)
    add("s1p", 0, 64, 128)
    add("s1o", 64, 64, 128)
    add("s1s", 0, 64, 128)
    add("s1t", 64, 64, 128)
    add("s2p", 0, 128, 1)
    add("s2o", 0, 128, 1)
    add("s2s", 0, 128, 1)
    add("s2t", 0, 128, 1)
    return spec, col


WSPEC, WCOL = _build_wspec()

# param blob columns (f32)
PB_L1G = 0      # cols 0-3: layer1 gate biases (i,f,g,o)
PB_L2G = 4      # cols 4-7
PB_E1 = {"physical": 8, "orbital": 9, "signature": 10}
PB_ZA = 11      # [b2_phys ; b2_orb]
PB_ZB = 12      # [b2_sig ; b_tp]
PB_FLOW = 13    # 13,14,15
PB_D1 = 16      # 16-19 dec l1 biases (p,o,s,t)
PB_S1 = 20      # 20-23 score l1 biases
PB_D2 = {"physical": 24, "orbital": 25, "signature": 26, "temporal": 28}  # sig uses 26,27
PB_S2 = 29      # 29-32, row 0 only
PB_LDW = 33     # rows 0-1: sum of flow log|det W|
PCOL = 34


def _np(x):
    return np.asarray(x, dtype=np.float32)


def build_blobs(params):
    """Pack all model parameters into WBLOB (bf16) and PBLOB (f32)."""
    wb = np.zeros((128, WCOL), np.float32)
    pb = np.zeros((128, PCOL), np.float32)

    def put(name, arr):
        r0, nr, c0, ncol = WSPEC[name]
        a = _np(arr)
        assert a.shape == (nr, ncol), (name, a.shape, (nr, ncol))
        wb[r0:r0 + nr, c0:c0 + ncol] = a

    lstm = params["lstm"]
    # layer 1 Wih replicated at row groups 0..3 (16 rows each, 32-aligned)
    wih1 = np.zeros((128, 512), np.float32)
    w = _np(lstm[0]["Wih"])  # [16, 512]
    for r in range(4):
        wih1[32 * r:32 * r + 16, :] = w
    put("wih1", wih1)
    put("whh1", _np(lstm[0]["Whh"]))
    put("wih2", _np(lstm[1]["Wih"]))
    put("whh2", _np(lstm[1]["Whh"]))
    put("wtp", _np(params["temporal_proj"]["W"]))
    put("e1p", _np(params["enc_physical"]["l1"]["W"]))
    put("e2p", _np(params["enc_physical"]["l2"]["W"]))
    put("e1o", _np(params["enc_orbital"]["l1"]["W"]))
    put("e2o", _np(params["enc_orbital"]["l2"]["W"]))
    ws = _np(params["enc_signature"]["l1"]["W"])  # [256, 128]
    put("e1s_a", ws[0:128])
    put("e1s_b", ws[128:256])
    put("e2s", _np(params["enc_signature"]["l2"]["W"]))
    for f in range(3):
        blk = np.zeros((128, 128), np.float32)
        fw = _np(params["flows"][f]["W"])
        blk[0:64, 0:64] = fw
        blk[64:128, 64:128] = fw
        put(f"fw{f}", blk)
    ones2 = np.zeros((128, 2), np.float32)
    ones2[0:64, 0] = 1.0
    ones2[64:128, 1] = 1.0
    put("ones2", ones2)
    key = {"physical": "p", "orbital": "o", "signature": "s", "temporal": "t"}
    for m in MODS:
        k = key[m]
        put(f"d1{k}", _np(params["dec_" + m]["l1"]["W"]))
        put(f"d2{k}", _np(params["dec_" + m]["l2"]["W"]))
        put(f"s1{k}", _np(params["score_" + m]["l1"]["W"]))
        put(f"s2{k}", _np(params["score_" + m]["l2"]["W"]))

    # ---- param blob ----
    b1 = _np(lstm[0]["bih"]) + _np(lstm[0]["bhh"])  # [512]
    b2 = _np(lstm[1]["bih"]) + _np(lstm[1]["bhh"])
    for g in range(4):
        pb[:, PB_L1G + g] = b1[g * 128:(g + 1) * 128]
        pb[:, PB_L2G + g] = b2[g * 128:(g + 1) * 128]
    pb[:, PB_E1["physical"]] = _np(params["enc_physical"]["l1"]["b"])
    pb[:, PB_E1["orbital"]] = _np(params["enc_orbital"]["l1"]["b"])
    pb[:, PB_E1["signature"]] = _np(params["enc_signature"]["l1"]["b"])
    pb[0:64, PB_ZA] = _np(params["enc_physical"]["l2"]["b"])
    pb[64:128, PB_ZA] = _np(params["enc_orbital"]["l2"]["b"])
    pb[0:64, PB_ZB] = _np(params["enc_signature"]["l2"]["b"])
    pb[64:128, PB_ZB] = _np(params["temporal_proj"]["b"])
    for f in range(3):
        bf = _np(params["flows"][f]["b"])
        pb[0:64, PB_FLOW + f] = bf
        pb[64:128, PB_FLOW + f] = bf
    for mi, m in enumerate(MODS):
        pb[:, PB_D1 + mi] = _np(params["dec_" + m]["l1"]["b"])
        pb[:, PB_S1 + mi] = _np(params["score_" + m]["l1"]["b"])
        pb[0, PB_S2 + mi] = _np(params["score_" + m]["l2"]["b"])[0]
    pb[0:32, PB_D2["physical"]] = _np(params["dec_physical"]["l2"]["b"])
    pb[0:6, PB_D2["orbital"]] = _np(params["dec_orbital"]["l2"]["b"])
    bs = _np(params["dec_signature"]["l2"]["b"])
    pb[:, PB_D2["signature"]] = bs[0:128]
    pb[:, PB_D2["signature"] + 1] = bs[128:256]
    pb[0:16, PB_D2["temporal"]] = _np(params["dec_temporal"]["l2"]["b"])

    sum_lw = 0.0
    for f in range(3):
        sum_lw += np.linalg.slogdet(_np(params["flows"][f]["W"]))[1]
    pb[0:2, PB_LDW] = sum_lw

    return wb.astype(ml_dtypes.bfloat16), pb


# ==================================================================
# Device program
# ==================================================================
def build_program(t_steps=T):
    nc = bacc.Bacc("TRN2", target_bir_lowering=False, debug=False,
                   num_devices=NCORES)

    XT = nc.declare_dram_parameter("xt", [16, 128, S], BF16, isOutput=False)
    XSIG = nc.declare_dram_parameter("xsig", [2, 128, S], BF16, isOutput=False)
    XPH = nc.declare_dram_parameter("xph", [32, S], BF16, isOutput=False)
    XOR = nc.declare_dram_parameter("xor", [6, S], BF16, isOutput=False)
    WB = nc.declare_dram_parameter("wb", [128, WCOL], BF16, isOutput=False)
    PBIN = nc.declare_dram_parameter("pbin", [128, PCOL], F32, isOutput=False)
    OUT = nc.declare_dram_parameter("out", [NROWS, S], F32, isOutput=True)

    from contextlib import ExitStack
    with TileContext(nc) as tc, ExitStack() as ctx:
        wpool = ctx.enter_context(tc.tile_pool(name="wpool", bufs=1))
        xpool = ctx.enter_context(tc.tile_pool(name="xpool", bufs=6))
        spool = ctx.enter_context(tc.tile_pool(name="spool", bufs=2))
        gpool = ctx.enter_context(tc.tile_pool(name="gpool", bufs=2))
        bpool = ctx.enter_context(tc.tile_pool(name="bpool", bufs=2))
        fpool = ctx.enter_context(tc.tile_pool(name="fpool", bufs=2))
        pspool = ctx.enter_context(tc.tile_pool(name="pspool", bufs=2, space="PSUM"))

        wb = wpool.tile([128, WCOL], BF16, tag="wb")
        pb = wpool.tile([128, PCOL], F32, tag="pb")
        nc.sync.dma_start(out=wb[:], in_=WB[:])
        nc.sync.dma_start(out=pb[:], in_=PBIN[:])

        def wap(name):
            r0, nr, c0, ncol = WSPEC[name]
            return wb[r0:r0 + nr, c0:c0 + ncol]

        def bias(col):
            return pb[:, col:col + 1]

        def mm_chain(ps_ap, lhsT, rhs, start=True, stop=True, tile_position=None):
            """matmul over all NCH sample chunks"""
            for n in range(NCH):
                nc.tensor.matmul(ps_ap[:, ts(n, CH)], lhsT, rhs[:, ts(n, CH)],
                                 start=start, stop=stop,
                                 tile_position=tile_position)

        # ---------------- pair block: enc-out, flows, dec, score -----------
        def emit_pair(pair, zps, mods):
            """zps: psum tile [128, S] holding the two encodings (pre-bias).
            mods: [(mod_name, global_mod_index), ...] for partitions 0-63 / 64-127."""
            zf = fpool.tile([128, S], F32, tag="zf")
            nc.vector.tensor_scalar_add(zf[:], zps[:], bias(PB_ZA + pair))
            nc.sync.dma_start(out=OUT[ROW_ENC + 128 * pair:ROW_ENC + 128 * (pair + 1), :],
                              in_=zf[:])
            zb = bpool.tile([128, S], BF16, tag="zb")
            nc.vector.tensor_copy(out=zb[:], in_=zf[:])

            z2s = []
            for f in range(3):
                ps = pspool.tile([128, S], F32, tag="ps")
                mm_chain(ps, wap(f"fw{f}"), zb)
                zb2 = bpool.tile([128, S], BF16, tag="zb")
                nc.scalar.activation(out=zb2[:], in_=ps[:], func=AF.Tanh,
                                     bias=bias(PB_FLOW + f))
                z2 = bpool.tile([128, S], BF16, tag=f"z2{f}", bufs=1)
                nc.vector.tensor_mul(z2[:], zb2[:], zb2[:])
                z2s.append(z2)
                zb = zb2

            trf = fpool.tile([128, S], F32, tag="zf")
            nc.vector.tensor_copy(out=trf[:], in_=zb[:])
            nc.sync.dma_start(out=OUT[ROW_TRF + 128 * pair:ROW_TRF + 128 * (pair + 1), :],
                              in_=trf[:])

            # log-det: sum_k log1p(-z^2) accumulated over flows then + sum_lw
            for f in range(3):
                nc.scalar.activation(out=z2s[f][:], in_=z2s[f][:], func=AF.Ln,
                                     bias=1.0, scale=-1.0)
            nc.vector.tensor_add(z2s[0][:], z2s[0][:], z2s[1][:])
            nc.vector.tensor_add(z2s[0][:], z2s[0][:], z2s[2][:])
            psl = pspool.tile([128, S], F32, tag="ps")
            mm_chain(psl[0:2, :], wap("ones2"), z2s[0])
            ldf = fpool.tile([2, S], F32, tag="ost")
            nc.vector.tensor_scalar_add(ldf[:], psl[0:2, :],
                                        pb[0:2, PB_LDW:PB_LDW + 1])
            nc.sync.dma_start(out=OUT[ROW_LD + 2 * pair:ROW_LD + 2 * pair + 2, :],
                              in_=ldf[:])

            # decoders + scores
            key = {"physical": "p", "orbital": "o", "signature": "s",
                   "temporal": "t"}
            for mi, (m, gmi) in enumerate(mods):
                k = key[m]
                zin = zb[64 * mi:64 * (mi + 1), :]
                # dec l1
                ps = pspool.tile([128, S], F32, tag="ps")
                mm_chain(ps, wap(f"d1{k}"), zin)
                hd = bpool.tile([128, S], BF16, tag="mh")
                nc.scalar.activation(out=hd[:], in_=ps[:], func=AF.Relu,
                                     bias=bias(PB_D1 + gmi))
                # dec l2 (chunks of <=128 output rows)
                D = DIMS[m]
                r0 = REC_OFF[m]
                nchunk = (D + 127) // 128
                for ci in range(nchunk):
                    nr = min(128, D - 128 * ci)
                    _, _, c0, _ = WSPEC[f"d2{k}"]
                    lhsT = wb[0:128, c0 + 128 * ci:c0 + 128 * ci + nr]
                    ps2 = pspool.tile([128, S], F32, tag="ps")
                    mm_chain(ps2[0:nr, :], lhsT, hd)
                    ost = fpool.tile([128, S], F32, tag="ost")
                    nc.vector.tensor_scalar_add(
                        ost[0:nr, :], ps2[0:nr, :],
                        pb[0:nr, PB_D2[m] + ci:PB_D2[m] + ci + 1])
                    nc.sync.dma_start(out=OUT[r0 + 128 * ci:r0 + 128 * ci + nr, :],
                                      in_=ost[0:nr, :])
                # score l1
                ps3 = pspool.tile([128, S], F32, tag="ps")
                mm_chain(ps3, wap(f"s1{k}"), zin)
                hs = bpool.tile([128, S], BF16, tag="mh")
                nc.scalar.activation(out=hs[:], in_=ps3[:], func=AF.Relu,
                                     bias=bias(PB_S1 + gmi))
                # score l2 + sigmoid
                ps4 = pspool.tile([128, S], F32, tag="ps")
                mm_chain(ps4[0:1, :], wap(f"s2{k}"), hs)
                scf = fpool.tile([1, S], F32, tag="ost")
                nc.scalar.activation(out=scf[:], in_=ps4[0:1, :], func=AF.Sigmoid,
                                     bias=pb[0:1, PB_S2 + gmi:PB_S2 + gmi + 1])
                nc.sync.dma_start(out=OUT[ROW_SC + gmi:ROW_SC + gmi + 1, :],
                                  in_=scf[:])

        # ---------------- pair A prelude: phys + orb encoders ----------------
        xph = bpool.tile([32, S], BF16, tag="xin")
        nc.sync.dma_start(out=xph[:], in_=XPH[:])
        xorb = bpool.tile([6, S], BF16, tag="xin")
        nc.sync.dma_start(out=xorb[:], in_=XOR[:])

        psp = pspool.tile([128, S], F32, tag="ps")
        mm_chain(psp, wap("e1p"), xph)
        hp = bpool.tile([128, S], BF16, tag="mh")
        nc.scalar.activation(out=hp[:], in_=psp[:], func=AF.Relu,
                             bias=bias(PB_E1["physical"]))
        zpsA = pspool.tile([128, S], F32, tag="ps")
        mm_chain(zpsA[0:64, :], wap("e2p"), hp)

        pso = pspool.tile([128, S], F32, tag="ps")
        mm_chain(pso, wap("e1o"), xorb)
        ho = bpool.tile([128, S], BF16, tag="mh")
        nc.scalar.activation(out=ho[:], in_=pso[:], func=AF.Relu,
                             bias=bias(PB_E1["orbital"]))
        mm_chain(zpsA[64:128, :], wap("e2o"), ho, tile_position=(0, 64))

        emit_pair(0, zpsA, [("physical", 0), ("orbital", 1)])

        # ---------------- LSTM ----------------
        h1 = spool.tile([128, S], BF16, tag="h1")
        c1 = spool.tile([128, S], BF16, tag="c1")
        h2 = spool.tile([128, S], BF16, tag="h2")
        c2 = spool.tile([128, S], BF16, tag="c2")
        for t0 in (h1, c1, h2, c2):
            nc.vector.memset(t0[:], 0.0)

        def lstm_layer(x_ap, wih_name, wih_r0, whh_name, bias0, prev_h, prev_c,
                       h_tag, c_tag):
            gates = {}
            for gi, nm in enumerate("ifgo"):
                ps = pspool.tile([128, S], F32, tag="ps")
                _, _, cih, _ = WSPEC[wih_name]
                _, _, chh, _ = WSPEC[whh_name]
                kin = x_ap.shape[0]
                lih = wb[wih_r0:wih_r0 + kin, cih + 128 * gi:cih + 128 * (gi + 1)]
                lhh = wb[0:128, chh + 128 * gi:chh + 128 * (gi + 1)]
                tp_ih = (96, 0) if wih_r0 == 96 else None
                for n in range(NCH):
                    nsl = ts(n, CH)
                    nc.tensor.matmul(ps[:, nsl], lih, x_ap[:, nsl],
                                     start=True, stop=False,
                                     tile_position=tp_ih)
                    nc.tensor.matmul(ps[:, nsl], lhh, prev_h[:, nsl],
                                     start=False, stop=True)
                g_sb = gpool.tile([128, S], BF16, tag="g" + nm)
                nc.scalar.activation(out=g_sb[:], in_=ps[:],
                                     func=AF.Tanh if nm == "g" else AF.Sigmoid,
                                     bias=bias(bias0 + gi))
                gates[nm] = g_sb
            nc.vector.tensor_mul(gates["f"][:], gates["f"][:], prev_c[:])
            nc.vector.tensor_mul(gates["i"][:], gates["i"][:], gates["g"][:])
            c_new = spool.tile([128, S], BF16, tag=c_tag)
            nc.vector.tensor_add(c_new[:], gates["f"][:], gates["i"][:])
            tc_sb = gpool.tile([128, S], BF16, tag="tc")
            nc.scalar.activation(out=tc_sb[:], in_=c_new[:], func=AF.Tanh)
            h_new = spool.tile([128, S], BF16, tag=h_tag)
            nc.vector.tensor_mul(h_new[:], gates["o"][:], tc_sb[:])
            return h_new, c_new

        xt_tile = None
        for t in range(t_steps):
            j, r = divmod(t, 4)
            if r == 0:
                xt_tile = xpool.tile([128, S], BF16, tag="xt")
                nc.sync.dma_start(out=xt_tile[:], in_=XT[j])
            x_ap = xt_tile[32 * r:32 * r + 16, :]
            h1, c1 = lstm_layer(x_ap, "wih1", 32 * r, "whh1", PB_L1G,
                                h1, c1, "h1", "c1")
            h2, c2 = lstm_layer(h1, "wih2", 0, "whh2", PB_L2G,
                                h2, c2, "h2", "c2")

        # ---------------- pair B: signature + temporal ----------------
        xs0 = bpool.tile([128, S], BF16, tag="xin")
        xs1 = bpool.tile([128, S], BF16, tag="xin")
        nc.sync.dma_start(out=xs0[:], in_=XSIG[0])
        nc.sync.dma_start(out=xs1[:], in_=XSIG[1])
        pss = pspool.tile([128, S], F32, tag="ps")
        for n in range(NCH):
            nsl = ts(n, CH)
            nc.tensor.matmul(pss[:, nsl], wap("e1s_a"), xs0[:, nsl],
                             start=True, stop=False)
            nc.tensor.matmul(pss[:, nsl], wap("e1s_b"), xs1[:, nsl],
                             start=False, stop=True)
        hsg = bpool.tile([128, S], BF16, tag="mh")
        nc.scalar.activation(out=hsg[:], in_=pss[:], func=AF.Relu,
                             bias=bias(PB_E1["signature"]))
        zpsB = pspool.tile([128, S], F32, tag="ps")
        mm_chain(zpsB[0:64, :], wap("e2s"), hsg)
        mm_chain(zpsB[64:128, :], wap("wtp"), h2, tile_position=(0, 64))

        emit_pair(1, zpsB, [("signature", 2), ("temporal", 3)])

    nc.compile()
    return nc


# ==================================================================
# Host wrapper
# ==================================================================
_CACHE = {}


def _prep_core_inputs(x_physical, x_orbital, x_signature, x_temporal, wb, pbin):
    bf16 = ml_dtypes.bfloat16
    in_maps = []
    for c in range(NCORES):
        sl = slice(c * S, (c + 1) * S)
        xt = np.transpose(x_temporal[sl], (1, 2, 0))          # [64, 16, S]
        xt = np.ascontiguousarray(xt).reshape(16, 4, 16, S)
        pad = np.zeros((16, 4, 32, S), np.float32)
        pad[:, :, :16, :] = xt
        XTc = pad.reshape(16, 128, S).astype(bf16)
        XSIGc = np.ascontiguousarray(x_signature[sl].T).reshape(2, 128, S).astype(bf16)
        XPHc = np.ascontiguousarray(x_physical[sl].T).astype(bf16)
        XORc = np.ascontiguousarray(x_orbital[sl].T).astype(bf16)
        in_maps.append({
            "xt": XTc, "xsig": XSIGc, "xph": XPHc, "xor": XORc,
            "wb": wb, "pbin": pbin,
        })
    return in_maps


LAST_RESULT = None


def kernel(x_physical, x_orbital, x_signature, x_temporal, params,
           _trace=False, _trace_kwargs=None):
    global LAST_RESULT
    x_physical = _np(x_physical)
    x_orbital = _np(x_orbital)
    x_signature = _np(x_signature)
    x_temporal = _np(x_temporal)

    wb, pbin = build_blobs(params)

    key = "prog"
    if key not in _CACHE:
        _CACHE[key] = build_program()
    nc = _CACHE[key]

    in_maps = _prep_core_inputs(x_physical, x_orbital, x_signature, x_temporal,
                                wb, pbin)
    res = run_bass_kernel_spmd(nc, in_maps, list(range(NCORES)),
                               trace=_trace, **(_trace_kwargs or {}))
    LAST_RESULT = res

    O = np.concatenate([res.results[c]["out"] for c in range(NCORES)], axis=1)

    def rows(r0, n):
        return np.ascontiguousarray(O[r0:r0 + n].T)

    enc = {}
    trf = {}
    rec = {}
    ld = {}
    sc = {}
    for mi, m in enumerate(MODS):
        enc[m] = rows(ROW_ENC + 64 * mi, 64)
        trf[m] = rows(ROW_TRF + 64 * mi, 64)
        rec[m] = rows(REC_OFF[m], DIMS[m])
        ld[m] = np.ascontiguousarray(O[ROW_LD + mi])
        sc[m] = rows(ROW_SC + mi, 1)
    return {
        "encodings": enc,
        "transformed": trf,
        "reconstructions": rec,
        "log_det": ld,
        "anomaly_scores": sc,
    }


# revision 6
# speedup vs baseline: 1.0628x; 1.0628x over previous
"""Trainium2 Bass kernel for nn_AnomalyDetector (multi-modal encoder + 2-layer
LSTM + normalizing flows + decoders + anomaly scores).

Data-parallel over 8 NeuronCores: batch 16384 -> 2048 per core. All on-chip
activations use a transposed layout [feature_on_partition, sample_on_free] so
the per-step LSTM matmuls need no transposes. Compute in bf16 with f32 PSUM
accumulation; outputs staged to one [830, 2048] f32 DRAM tensor per core.
"""

import numpy as np
import ml_dtypes

import concourse.bass as bass
import concourse.mybir as mybir
from concourse import bacc
from concourse.tile import TileContext
from concourse.bass import ts
from concourse.bass_utils import run_bass_kernel_spmd

BF16 = mybir.dt.bfloat16
F32 = mybir.dt.float32
AF = mybir.ActivationFunctionType

B = 16384
NCORES = 8
S = B // NCORES          # samples per core (2048)
T = 64
HID = 128
LAT = 64
DIMS = {"physical": 32, "orbital": 6, "signature": 256, "temporal": 16}
MODS = ["physical", "orbital", "signature", "temporal"]
NCH = 4                  # moving-sample chunks of 512
CH = S // NCH            # 512

# ------------------------------------------------------------------
# Output row map for the per-core OUT tensor [830, S] (feature, sample)
# ------------------------------------------------------------------
ROW_ENC = 0        # 4 x 64  (phys, orb, sig, temp)
ROW_TRF = 256      # 4 x 64
ROW_REC = 512      # 32, 6, 256, 16
REC_OFF = {"physical": 512, "orbital": 544, "signature": 550, "temporal": 806}
ROW_LD = 822       # 4 rows (phys, orb, sig, temp)
ROW_SC = 826       # 4 rows
NROWS = 830

# ------------------------------------------------------------------
# Weight blob (bf16) / param blob (f32) layouts: name -> (r0, nr, c0, nc)
# ------------------------------------------------------------------
def _build_wspec():
    spec = {}
    col = 0

    def add(name, r0, nr, nc):
        nonlocal col
        spec[name] = (r0, nr, col, nc)
        col += nc

    add("wih1", 0, 128, 512)      # 4x replicated along 32-row groups
    add("whh1", 0, 128, 512)
    add("wih2", 0, 128, 512)
    add("whh2", 0, 128, 512)
    add("wtp", 0, 128, 64)
    add("e1p", 0, 32, 128)
    add("e2p", 0, 128, 64)
    add("e1o", 0, 6, 128)
    add("e2o", 0, 128, 64)
    add("e1s_a", 0, 128, 128)
    add("e1s_b", 0, 128, 128)
    add("e2s", 0, 128, 64)
    add("fw0", 0, 128, 128)
    add("fw1", 0, 128, 128)
    add("fw2", 0, 128, 128)
    add("ones2", 0, 128, 2)
    add("d1p", 0, 64, 128)
    add("d1o", 64, 64, 128)
    add("d1s", 0, 64, 128)
    add("d1t", 64, 64, 128)
    add("d2p", 0, 128, 32)
    add("d2o", 0, 128, 6)
    add("d2s", 0, 128, 256)
    add("d2t", 0, 128, 16)
    add("s1p", 0, 64, 128)
    add("s1o", 64, 64, 128)
    add("s1s", 0, 64, 128)
    add("s1t", 64, 64, 128)
    add("s2p", 0, 128, 1)
    add("s2o", 0, 128, 1)
    add("s2s", 0, 128, 1)
    add("s2t", 0, 128, 1)
    return spec, col


WSPEC, WCOL = _build_wspec()

# param blob columns (f32)
PB_L1G = 0      # cols 0-3: layer1 gate biases (i,f,g,o)
PB_L2G = 4      # cols 4-7
PB_E1 = {"physical": 8, "orbital": 9, "signature": 10}
PB_ZA = 11      # [b2_phys ; b2_orb]
PB_ZB = 12      # [b2_sig ; b_tp]
PB_FLOW = 13    # 13,14,15
PB_D1 = 16      # 16-19 dec l1 biases (p,o,s,t)
PB_S1 = 20      # 20-23 score l1 biases
PB_D2 = {"physical": 24, "orbital": 25, "signature": 26, "temporal": 28}  # sig uses 26,27
PB_S2 = 29      # 29-32, row 0 only
PB_LDW = 33     # rows 0-1: sum of flow log|det W|
PCOL = 34


def _np(x):
    return np.asarray(x, dtype=np.float32)


def build_blobs(params):
    """Pack all model parameters into WBLOB (bf16) and PBLOB (f32)."""
    wb = np.zeros((128, WCOL), np.float32)
    pb = np.zeros((128, PCOL), np.float32)

    def put(name, arr):
        r0, nr, c0, ncol = WSPEC[name]
        a = _np(arr)
        assert a.shape == (nr, ncol), (name, a.shape, (nr, ncol))
        wb[r0:r0 + nr, c0:c0 + ncol] = a

    lstm = params["lstm"]
    # layer 1 Wih replicated at row groups 0..3 (16 rows each, 32-aligned)
    wih1 = np.zeros((128, 512), np.float32)
    w = _np(lstm[0]["Wih"])  # [16, 512]
    for r in range(4):
        wih1[32 * r:32 * r + 16, :] = w
    put("wih1", wih1)
    put("whh1", _np(lstm[0]["Whh"]))
    put("wih2", _np(lstm[1]["Wih"]))
    put("whh2", _np(lstm[1]["Whh"]))
    put("wtp", _np(params["temporal_proj"]["W"]))
    put("e1p", _np(params["enc_physical"]["l1"]["W"]))
    put("e2p", _np(params["enc_physical"]["l2"]["W"]))
    put("e1o", _np(params["enc_orbital"]["l1"]["W"]))
    put("e2o", _np(params["enc_orbital"]["l2"]["W"]))
    ws = _np(params["enc_signature"]["l1"]["W"])  # [256, 128]
    put("e1s_a", ws[0:128])
    put("e1s_b", ws[128:256])
    put("e2s", _np(params["enc_signature"]["l2"]["W"]))
    for f in range(3):
        blk = np.zeros((128, 128), np.float32)
        fw = _np(params["flows"][f]["W"])
        blk[0:64, 0:64] = fw
        blk[64:128, 64:128] = fw
        put(f"fw{f}", blk)
    ones2 = np.zeros((128, 2), np.float32)
    ones2[0:64, 0] = 1.0
    ones2[64:128, 1] = 1.0
    put("ones2", ones2)
    key = {"physical": "p", "orbital": "o", "signature": "s", "temporal": "t"}
    for m in MODS:
        k = key[m]
        put(f"d1{k}", _np(params["dec_" + m]["l1"]["W"]))
        put(f"d2{k}", _np(params["dec_" + m]["l2"]["W"]))
        put(f"s1{k}", _np(params["score_" + m]["l1"]["W"]))
        put(f"s2{k}", _np(params["score_" + m]["l2"]["W"]))

    # ---- param blob ----
    b1 = _np(lstm[0]["bih"]) + _np(lstm[0]["bhh"])  # [512]
    b2 = _np(lstm[1]["bih"]) + _np(lstm[1]["bhh"])
    for g in range(4):
        pb[:, PB_L1G + g] = b1[g * 128:(g + 1) * 128]
        pb[:, PB_L2G + g] = b2[g * 128:(g + 1) * 128]
    pb[:, PB_E1["physical"]] = _np(params["enc_physical"]["l1"]["b"])
    pb[:, PB_E1["orbital"]] = _np(params["enc_orbital"]["l1"]["b"])
    pb[:, PB_E1["signature"]] = _np(params["enc_signature"]["l1"]["b"])
    pb[0:64, PB_ZA] = _np(params["enc_physical"]["l2"]["b"])
    pb[64:128, PB_ZA] = _np(params["enc_orbital"]["l2"]["b"])
    pb[0:64, PB_ZB] = _np(params["enc_signature"]["l2"]["b"])
    pb[64:128, PB_ZB] = _np(params["temporal_proj"]["b"])
    for f in range(3):
        bf = _np(params["flows"][f]["b"])
        pb[0:64, PB_FLOW + f] = bf
        pb[64:128, PB_FLOW + f] = bf
    for mi, m in enumerate(MODS):
        pb[:, PB_D1 + mi] = _np(params["dec_" + m]["l1"]["b"])
        pb[:, PB_S1 + mi] = _np(params["score_" + m]["l1"]["b"])
        pb[0, PB_S2 + mi] = _np(params["score_" + m]["l2"]["b"])[0]
    pb[0:32, PB_D2["physical"]] = _np(params["dec_physical"]["l2"]["b"])
    pb[0:6, PB_D2["orbital"]] = _np(params["dec_orbital"]["l2"]["b"])
    bs = _np(params["dec_signature"]["l2"]["b"])
    pb[:, PB_D2["signature"]] = bs[0:128]
    pb[:, PB_D2["signature"] + 1] = bs[128:256]
    pb[0:16, PB_D2["temporal"]] = _np(params["dec_temporal"]["l2"]["b"])

    sum_lw = 0.0
    for f in range(3):
        sum_lw += np.linalg.slogdet(_np(params["flows"][f]["W"]))[1]
    pb[0:2, PB_LDW] = sum_lw

    return wb.astype(ml_dtypes.bfloat16), pb


# ==================================================================
# Device program
# ==================================================================
def build_program(t_steps=T):
    nc = bacc.Bacc("TRN2", target_bir_lowering=False, debug=False,
                   num_devices=NCORES)

    XT = nc.declare_dram_parameter("xt", [16, 128, S], BF16, isOutput=False)
    XSIG = nc.declare_dram_parameter("xsig", [2, 128, S], BF16, isOutput=False)
    XPH = nc.declare_dram_parameter("xph", [32, S], BF16, isOutput=False)
    XOR = nc.declare_dram_parameter("xor", [6, S], BF16, isOutput=False)
    WB = nc.declare_dram_parameter("wb", [128, WCOL], BF16, isOutput=False)
    PBIN = nc.declare_dram_parameter("pbin", [128, PCOL], F32, isOutput=False)
    OUT = nc.declare_dram_parameter("out", [NROWS, S], F32, isOutput=True)

    from contextlib import ExitStack
    with TileContext(nc) as tc, ExitStack() as ctx:
        wpool = ctx.enter_context(tc.tile_pool(name="wpool", bufs=1))
        xpool = ctx.enter_context(tc.tile_pool(name="xpool", bufs=6))
        spool = ctx.enter_context(tc.tile_pool(name="spool", bufs=2))
        gpool = ctx.enter_context(tc.tile_pool(name="gpool", bufs=2))
        bpool = ctx.enter_context(tc.tile_pool(name="bpool", bufs=2))
        fpool = ctx.enter_context(tc.tile_pool(name="fpool", bufs=2))
        pspool = ctx.enter_context(tc.tile_pool(name="pspool", bufs=2, space="PSUM"))

        wb = wpool.tile([128, WCOL], BF16, tag="wb")
        pb = wpool.tile([128, PCOL], F32, tag="pb")
        nc.sync.dma_start(out=wb[:], in_=WB[:])
        nc.sync.dma_start(out=pb[:], in_=PBIN[:])

        def wap(name):
            r0, nr, c0, ncol = WSPEC[name]
            return wb[r0:r0 + nr, c0:c0 + ncol]

        def bias(col):
            return pb[:, col:col + 1]

        def mm_chain(ps_ap, lhsT, rhs, start=True, stop=True, tile_position=None):
            """matmul over all NCH sample chunks"""
            for n in range(NCH):
                nc.tensor.matmul(ps_ap[:, ts(n, CH)], lhsT, rhs[:, ts(n, CH)],
                                 start=start, stop=stop,
                                 tile_position=tile_position)

        # ---------------- pair block: enc-out, flows, dec, score -----------
        def emit_pair(pair, zps, mods):
            """zps: psum tile [128, S] holding the two encodings (pre-bias).
            mods: [(mod_name, global_mod_index), ...] for partitions 0-63 / 64-127."""
            zf = fpool.tile([128, S], F32, tag="zf")
            nc.vector.tensor_scalar_add(zf[:], zps[:], bias(PB_ZA + pair))
            nc.sync.dma_start(out=OUT[ROW_ENC + 128 * pair:ROW_ENC + 128 * (pair + 1), :],
                              in_=zf[:])
            zb = bpool.tile([128, S], BF16, tag="zb")
            nc.vector.tensor_copy(out=zb[:], in_=zf[:])

            z2s = []
            for f in range(3):
                ps = pspool.tile([128, S], F32, tag="ps")
                mm_chain(ps, wap(f"fw{f}"), zb)
                zb2 = bpool.tile([128, S], BF16, tag="zb")
                nc.scalar.activation(out=zb2[:], in_=ps[:], func=AF.Tanh,
                                     bias=bias(PB_FLOW + f))
                z2 = bpool.tile([128, S], BF16, tag=f"z2{f}", bufs=1)
                nc.vector.tensor_mul(z2[:], zb2[:], zb2[:])
                z2s.append(z2)
                zb = zb2

            trf = fpool.tile([128, S], F32, tag="zf")
            nc.vector.tensor_copy(out=trf[:], in_=zb[:])
            nc.sync.dma_start(out=OUT[ROW_TRF + 128 * pair:ROW_TRF + 128 * (pair + 1), :],
                              in_=trf[:])

            # log-det: sum_k log1p(-z^2) accumulated over flows then + sum_lw
            for f in range(3):
                nc.scalar.activation(out=z2s[f][:], in_=z2s[f][:], func=AF.Ln,
                                     bias=1.0, scale=-1.0)
            nc.vector.tensor_add(z2s[0][:], z2s[0][:], z2s[1][:])
            nc.vector.tensor_add(z2s[0][:], z2s[0][:], z2s[2][:])
            psl = pspool.tile([128, S], F32, tag="ps")
            mm_chain(psl[0:2, :], wap("ones2"), z2s[0])
            ldf = fpool.tile([2, S], F32, tag="ost")
            nc.vector.tensor_scalar_add(ldf[:], psl[0:2, :],
                                        pb[0:2, PB_LDW:PB_LDW + 1])
            nc.sync.dma_start(out=OUT[ROW_LD + 2 * pair:ROW_LD + 2 * pair + 2, :],
                              in_=ldf[:])

            # decoders + scores
            key = {"physical": "p", "orbital": "o", "signature": "s",
                   "temporal": "t"}
            for mi, (m, gmi) in enumerate(mods):
                k = key[m]
                zin = zb[64 * mi:64 * (mi + 1), :]
                # dec l1
                ps = pspool.tile([128, S], F32, tag="ps")
                mm_chain(ps, wap(f"d1{k}"), zin)
                hd = bpool.tile([128, S], BF16, tag="mh")
                nc.scalar.activation(out=hd[:], in_=ps[:], func=AF.Relu,
                                     bias=bias(PB_D1 + gmi))
                # dec l2 (chunks of <=128 output rows)
                D = DIMS[m]
                r0 = REC_OFF[m]
                nchunk = (D + 127) // 128
                for ci in range(nchunk):
                    nr = min(128, D - 128 * ci)
                    _, _, c0, _ = WSPEC[f"d2{k}"]
                    lhsT = wb[0:128, c0 + 128 * ci:c0 + 128 * ci + nr]
                    ps2 = pspool.tile([128, S], F32, tag="ps")
                    mm_chain(ps2[0:nr, :], lhsT, hd)
                    ost = fpool.tile([128, S], F32, tag="ost")
                    nc.vector.tensor_scalar_add(
                        ost[0:nr, :], ps2[0:nr, :],
                        pb[0:nr, PB_D2[m] + ci:PB_D2[m] + ci + 1])
                    nc.sync.dma_start(out=OUT[r0 + 128 * ci:r0 + 128 * ci + nr, :],
                                      in_=ost[0:nr, :])
                # score l1
                ps3 = pspool.tile([128, S], F32, tag="ps")
                mm_chain(ps3, wap(f"s1{k}"), zin)
                hs = bpool.tile([128, S], BF16, tag="mh")
                nc.scalar.activation(out=hs[:], in_=ps3[:], func=AF.Relu,
                                     bias=bias(PB_S1 + gmi))
                # score l2 + sigmoid
                ps4 = pspool.tile([128, S], F32, tag="ps")
                mm_chain(ps4[0:1, :], wap(f"s2{k}"), hs)
                scf = fpool.tile([1, S], F32, tag="ost")
                nc.scalar.activation(out=scf[:], in_=ps4[0:1, :], func=AF.Sigmoid,
                                     bias=pb[0:1, PB_S2 + gmi:PB_S2 + gmi + 1])
                nc.sync.dma_start(out=OUT[ROW_SC + gmi:ROW_SC + gmi + 1, :],
                                  in_=scf[:])

        # ---------------- pair A prelude: phys + orb encoders ----------------
        xph = bpool.tile([32, S], BF16, tag="xin")
        nc.sync.dma_start(out=xph[:], in_=XPH[:])
        xorb = bpool.tile([6, S], BF16, tag="xin")
        nc.sync.dma_start(out=xorb[:], in_=XOR[:])

        psp = pspool.tile([128, S], F32, tag="ps")
        mm_chain(psp, wap("e1p"), xph)
        hp = bpool.tile([128, S], BF16, tag="mh")
        nc.scalar.activation(out=hp[:], in_=psp[:], func=AF.Relu,
                             bias=bias(PB_E1["physical"]))
        zpsA = pspool.tile([128, S], F32, tag="ps")
        mm_chain(zpsA[0:64, :], wap("e2p"), hp)

        pso = pspool.tile([128, S], F32, tag="ps")
        mm_chain(pso, wap("e1o"), xorb)
        ho = bpool.tile([128, S], BF16, tag="mh")
        nc.scalar.activation(out=ho[:], in_=pso[:], func=AF.Relu,
                             bias=bias(PB_E1["orbital"]))
        mm_chain(zpsA[64:128, :], wap("e2o"), ho, tile_position=(0, 64))

        emit_pair(0, zpsA, [("physical", 0), ("orbital", 1)])

        # ---------------- LSTM ----------------
        h1 = spool.tile([128, S], BF16, tag="h1")
        c1 = spool.tile([128, S], BF16, tag="c1")
        h2 = spool.tile([128, S], BF16, tag="h2")
        c2 = spool.tile([128, S], BF16, tag="c2")
        for t0 in (h1, c1, h2, c2):
            nc.vector.memset(t0[:], 0.0)

        def lstm_layer(x_ap, wih_name, wih_r0, whh_name, bias0, prev_h, prev_c,
                       h_tag, c_tag, hh_first):
            # hh_first: for layer 2, h2(t-1) is available long before h1(t),
            # so the Whh pass goes first (start=True) to keep the PE busy
            # while layer 1's cell-update chain completes.
            gates = {}
            for gi, nm in enumerate("ifgo"):
                ps = pspool.tile([128, S], F32, tag="ps")
                _, _, cih, _ = WSPEC[wih_name]
                _, _, chh, _ = WSPEC[whh_name]
                kin = x_ap.shape[0]
                lih = wb[wih_r0:wih_r0 + kin, cih + 128 * gi:cih + 128 * (gi + 1)]
                lhh = wb[0:128, chh + 128 * gi:chh + 128 * (gi + 1)]
                tp_ih = (96, 0) if wih_r0 == 96 else None
                for n in range(NCH):
                    nsl = ts(n, CH)
                    if hh_first:
                        nc.tensor.matmul(ps[:, nsl], lhh, prev_h[:, nsl],
                                         start=True, stop=False)
                        nc.tensor.matmul(ps[:, nsl], lih, x_ap[:, nsl],
                                         start=False, stop=True,
                                         tile_position=tp_ih)
                    else:
                        nc.tensor.matmul(ps[:, nsl], lih, x_ap[:, nsl],
                                         start=True, stop=False,
                                         tile_position=tp_ih)
                        nc.tensor.matmul(ps[:, nsl], lhh, prev_h[:, nsl],
                                         start=False, stop=True)
                g_sb = gpool.tile([128, S], BF16, tag="g" + nm)
                nc.scalar.activation(out=g_sb[:], in_=ps[:],
                                     func=AF.Tanh if nm == "g" else AF.Sigmoid,
                                     bias=bias(bias0 + gi))
                gates[nm] = g_sb
            # cell update in two sample halves so ACT/DVE pipeline
            c_new = spool.tile([128, S], BF16, tag=c_tag)
            tc_sb = gpool.tile([128, S], BF16, tag="tc")
            h_new = spool.tile([128, S], BF16, tag=h_tag)
            H = S // 2
            for hf in range(2):
                sl = ts(hf, H)
                nc.vector.tensor_mul(gates["f"][:, sl], gates["f"][:, sl],
                                     prev_c[:, sl])
                nc.vector.tensor_mul(gates["i"][:, sl], gates["i"][:, sl],
                                     gates["g"][:, sl])
                nc.vector.tensor_add(c_new[:, sl], gates["f"][:, sl],
                                     gates["i"][:, sl])
                nc.scalar.activation(out=tc_sb[:, sl], in_=c_new[:, sl],
                                     func=AF.Tanh)
                nc.vector.tensor_mul(h_new[:, sl], gates["o"][:, sl],
                                     tc_sb[:, sl])
            return h_new, c_new

        xt_tile = None
        for t in range(t_steps):
            j, r = divmod(t, 4)
            if r == 0:
                xt_tile = xpool.tile([128, S], BF16, tag="xt")
                nc.sync.dma_start(out=xt_tile[:], in_=XT[j])
            x_ap = xt_tile[32 * r:32 * r + 16, :]
            h1, c1 = lstm_layer(x_ap, "wih1", 32 * r, "whh1", PB_L1G,
                                h1, c1, "h1", "c1", hh_first=False)
            h2, c2 = lstm_layer(h1, "wih2", 0, "whh2", PB_L2G,
                                h2, c2, "h2", "c2", hh_first=True)

        # ---------------- pair B: signature + temporal ----------------
        xs0 = bpool.tile([128, S], BF16, tag="xin")
        xs1 = bpool.tile([128, S], BF16, tag="xin")
        nc.sync.dma_start(out=xs0[:], in_=XSIG[0])
        nc.sync.dma_start(out=xs1[:], in_=XSIG[1])
        pss = pspool.tile([128, S], F32, tag="ps")
        for n in range(NCH):
            nsl = ts(n, CH)
            nc.tensor.matmul(pss[:, nsl], wap("e1s_a"), xs0[:, nsl],
                             start=True, stop=False)
            nc.tensor.matmul(pss[:, nsl], wap("e1s_b"), xs1[:, nsl],
                             start=False, stop=True)
        hsg = bpool.tile([128, S], BF16, tag="mh")
        nc.scalar.activation(out=hsg[:], in_=pss[:], func=AF.Relu,
                             bias=bias(PB_E1["signature"]))
        zpsB = pspool.tile([128, S], F32, tag="ps")
        mm_chain(zpsB[0:64, :], wap("e2s"), hsg)
        mm_chain(zpsB[64:128, :], wap("wtp"), h2, tile_position=(0, 64))

        emit_pair(1, zpsB, [("signature", 2), ("temporal", 3)])

    nc.compile()
    return nc


# ==================================================================
# Host wrapper
# ==================================================================
_CACHE = {}


def _prep_core_inputs(x_physical, x_orbital, x_signature, x_temporal, wb, pbin):
    bf16 = ml_dtypes.bfloat16
    in_maps = []
    for c in range(NCORES):
        sl = slice(c * S, (c + 1) * S)
        xt = np.transpose(x_temporal[sl], (1, 2, 0))          # [64, 16, S]
        xt = np.ascontiguousarray(xt).reshape(16, 4, 16, S)
        pad = np.zeros((16, 4, 32, S), np.float32)
        pad[:, :, :16, :] = xt
        XTc = pad.reshape(16, 128, S).astype(bf16)
        XSIGc = np.ascontiguousarray(x_signature[sl].T).reshape(2, 128, S).astype(bf16)
        XPHc = np.ascontiguousarray(x_physical[sl].T).astype(bf16)
        XORc = np.ascontiguousarray(x_orbital[sl].T).astype(bf16)
        in_maps.append({
            "xt": XTc, "xsig": XSIGc, "xph": XPHc, "xor": XORc,
            "wb": wb, "pbin": pbin,
        })
    return in_maps


LAST_RESULT = None


def kernel(x_physical, x_orbital, x_signature, x_temporal, params,
           _trace=False, _trace_kwargs=None):
    global LAST_RESULT
    x_physical = _np(x_physical)
    x_orbital = _np(x_orbital)
    x_signature = _np(x_signature)
    x_temporal = _np(x_temporal)

    wb, pbin = build_blobs(params)

    key = "prog"
    if key not in _CACHE:
        _CACHE[key] = build_program()
    nc = _CACHE[key]

    in_maps = _prep_core_inputs(x_physical, x_orbital, x_signature, x_temporal,
                                wb, pbin)
    res = run_bass_kernel_spmd(nc, in_maps, list(range(NCORES)),
                               trace=_trace, **(_trace_kwargs or {}))
    LAST_RESULT = res

    O = np.concatenate([res.results[c]["out"] for c in range(NCORES)], axis=1)

    def rows(r0, n):
        return np.ascontiguousarray(O[r0:r0 + n].T)

    enc = {}
    trf = {}
    rec = {}
    ld = {}
    sc = {}
    for mi, m in enumerate(MODS):
        enc[m] = rows(ROW_ENC + 64 * mi, 64)
        trf[m] = rows(ROW_TRF + 64 * mi, 64)
        rec[m] = rows(REC_OFF[m], DIMS[m])
        ld[m] = np.ascontiguousarray(O[ROW_LD + mi])
        sc[m] = rows(ROW_SC + mi, 1)
    return {
        "encodings": enc,
        "transformed": trf,
        "reconstructions": rec,
        "log_det": ld,
        "anomaly_scores": sc,
    }
